# revision 1
# baseline (speedup 1.0000x reference)
"""JANet RNN kernel for Trainium2, 8-way data-parallel over batch.

Math (per batch row b):
    xz[t] = x[t] @ kernel + bias            # [2C], precomputed projection
    z[t]  = xz[t] + c[t-1] @ W              # W = recurrent_kernel [C, 2C]
    f     = sigmoid(z[t][:C]); g = tanh(z[t][C:])
    c[t]  = f * c[t-1] + (1 - f) * g
    out   = c[T-1] @ dense_w + dense_b

On-chip layout (per core, BL = 32 batch rows):
  - state kept transposed+packed: c[p, 32*ki + b] = c_state[cell 128*ki + p, b]
  - per-step PSUM tile z[128, 128]: cols 32*q..32*q+32 = z[cell chunk q][b],
    q = 0,1 -> forget-gate half, q = 2,3 -> candidate half.
  - z is built entirely by PE accumulation: bias matmul (K=4 against a
    one-hot selector) + 4 projection matmuls (kernel chunks vs pre-transposed
    x) + 8 recurrent matmuls (W chunks vs packed state, bf16).
  - ScalarE: sigmoid on z[:, :64], tanh on z[:, 64:]; VectorE: c update.

Inputs are pre-arranged on host (x transposed to [I, T, BL] bf16 per core, so
no on-chip transposes are needed anywhere).
"""

import numpy as np
import ml_dtypes

B, T, I, C, O = 256, 1024, 128, 256, 10
NCORES = 8
BL = B // NCORES          # 32 batch rows per core
TW = 16                   # timesteps per x DMA window
PG = 4                    # timesteps per projection/bias matmul group
ZB = 6                    # PSUM z-tile ring depth

_cache: dict = {}


def _build(t_steps: int):
    import concourse.mybir as mybir
    from concourse import bacc, tile

    dt = mybir.dt
    fp32, bf16 = dt.float32, dt.bfloat16
    AF = mybir.ActivationFunctionType

    from contextlib import ExitStack

    nc = bacc.Bacc(None, target_bir_lowering=False)
    names = {}

    with tile.TileContext(nc) as tc, ExitStack() as es:
        if True:
            dram = es.enter_context(tc.tile_pool(name="dram", bufs=1, space="DRAM"))
            xt = dram.tile([128, t_steps, BL], bf16, kind="ExternalInput", name="xt")
            wr = dram.tile([2, 128, 2 * C], bf16, kind="ExternalInput", name="wr")
            kr = dram.tile([128, 2 * C], bf16, kind="ExternalInput", name="kr")
            b4 = dram.tile([4, 128], fp32, kind="ExternalInput", name="b4")
            sel = dram.tile([4, 128], fp32, kind="ExternalInput", name="sel")
            dw = dram.tile([2, 128, O], fp32, kind="ExternalInput", name="dw")
            db = dram.tile([O, 1], fp32, kind="ExternalInput", name="db")
            yT = dram.tile([O, BL], fp32, kind="ExternalOutput", name="yT")
            names = {k: v.tensor.name for k, v in
                     dict(xt=xt, wr=wr, kr=kr, b4=b4, sel=sel, dw=dw, db=db, yT=yT).items()}

            cpool = es.enter_context(tc.tile_pool(name="consts", bufs=1))
            w0 = cpool.tile([128, 2 * C], bf16, tag="w0")
            w1 = cpool.tile([128, 2 * C], bf16, tag="w1")
            krs = cpool.tile([128, 2 * C], bf16, tag="krs")
            b4s = cpool.tile([4, 128], fp32, tag="b4s")
            sels = cpool.tile([4, 128], fp32, tag="sels")
            dw0 = cpool.tile([128, O], fp32, tag="dw0")
            dw1 = cpool.tile([128, O], fp32, tag="dw1")
            dbs = cpool.tile([O, 1], fp32, tag="dbs")
            cst = cpool.tile([128, 2 * BL], fp32, tag="cst")   # fp32 state
            cbf = cpool.tile([128, 2 * BL], bf16, tag="cbf")   # bf16 mirror for matmul rhs
            ysb = cpool.tile([O, BL], fp32, tag="ysb")

            nc.sync.dma_start(w0[:], wr[0])
            nc.sync.dma_start(w1[:], wr[1])
            nc.sync.dma_start(krs[:], kr[:])
            nc.sync.dma_start(b4s[:], b4[:])
            nc.sync.dma_start(sels[:], sel[:])
            nc.sync.dma_start(dw0[:], dw[0])
            nc.sync.dma_start(dw1[:], dw[1])
            nc.sync.dma_start(dbs[:], db[:])
            nc.vector.memset(cst[:], 0.0)
            nc.vector.memset(cbf[:], 0.0)

            xpool = es.enter_context(tc.tile_pool(name="xwin", bufs=3))
            zpool = es.enter_context(tc.tile_pool(name="zp", bufs=ZB, space="PSUM"))
            fpool = es.enter_context(tc.tile_pool(name="fp", bufs=2))
            gpool = es.enter_context(tc.tile_pool(name="gp", bufs=2))
            dpool = es.enter_context(tc.tile_pool(name="dp", bufs=2))
            epool = es.enter_context(tc.tile_pool(name="ep", bufs=2))

            wtiles = (w0, w1)
            xw = None
            zt = {}

            for t in range(t_steps):
                tl = t % TW
                if tl == 0:
                    tw = min(TW, t_steps - t)
                    xw = xpool.tile([128, TW * BL], bf16, tag="xw", name=f"xw{t}")
                    nc.sync.dma_start(xw[:, 0:tw * BL], xt[:, t:t + tw, :])

                if t % PG == 0:
                    # allocate z tiles & run bias+projection for steps t..t+PG-1
                    for j in range(t, min(t + PG, t_steps)):
                        zt[j] = zpool.tile([128, 128], fp32, tag="z", name=f"z{j}")
                        nc.tensor.matmul(zt[j][:], b4s[:], sels[:],
                                         start=True, stop=False)
                    for q in range(4):
                        for j in range(t, min(t + PG, t_steps)):
                            jl = j % TW
                            nc.tensor.matmul(
                                zt[j][:, 32 * q:32 * (q + 1)],
                                krs[:, 128 * q:128 * (q + 1)],
                                xw[:, BL * jl:BL * (jl + 1)],
                                start=False, stop=False)

                # recurrent matmuls: z[t] += W.T-chunks @ c
                for ki in range(2):
                    for q in range(4):
                        nc.tensor.matmul(
                            zt[t][:, 32 * q:32 * (q + 1)],
                            wtiles[ki][:, 128 * q:128 * (q + 1)],
                            cbf[:, BL * ki:BL * (ki + 1)],
                            start=False, stop=(ki == 1 and q == 3))

                f = fpool.tile([128, 2 * BL], fp32, tag="f", name=f"f{t}")
                g = gpool.tile([128, 2 * BL], fp32, tag="g", name=f"g{t}")
                nc.scalar.activation(f[:], zt[t][:, 0:2 * BL], AF.Sigmoid)
                nc.scalar.activation(g[:], zt[t][:, 2 * BL:4 * BL], AF.Tanh)

                d = dpool.tile([128, 2 * BL], fp32, tag="d", name=f"d{t}")
                e = epool.tile([128, 2 * BL], fp32, tag="e", name=f"e{t}")
                nc.vector.tensor_sub(d[:], cst[:], g[:])      # c - g
                nc.vector.tensor_mul(e[:], d[:], f[:])        # f*(c-g)
                nc.vector.tensor_add(cst[:], e[:], g[:])      # + g
                nc.vector.tensor_copy(cbf[:], cst[:])         # bf16 for next matmul
                del zt[t]

            # dense head: yT = dense_w.T @ c  (+ dense_b)
            ypool = es.enter_context(tc.tile_pool(name="yp", bufs=1, space="PSUM"))
            yp = ypool.tile([O, BL], fp32, tag="y")
            nc.tensor.matmul(yp[:], dw0[:], cst[:, 0:BL], start=True, stop=False)
            nc.tensor.matmul(yp[:], dw1[:], cst[:, BL:2 * BL], start=False, stop=True)
            nc.vector.tensor_scalar_add(ysb[:], yp[:], dbs[:])
            nc.sync.dma_start(yT[:], ysb[:])

    nc.compile()
    return nc, names


def _get_built(t_steps: int):
    key = t_steps
    if key not in _cache:
        _cache[key] = _build(t_steps)
    return _cache[key]


def kernel(x, kernel, recurrent_kernel, recurrent_bias, dense_w, dense_b,
           t_steps: int = T, _want_trace: bool = False):
    from concourse.bass_utils import run_bass_kernel_spmd

    bf = ml_dtypes.bfloat16
    nc, names = _get_built(t_steps)

    wr_np = np.ascontiguousarray(
        recurrent_kernel.astype(bf).reshape(2, 128, 2 * C))
    kr_np = np.ascontiguousarray(kernel.astype(bf))
    b4_np = np.ascontiguousarray(recurrent_bias.astype(np.float32).reshape(4, 128))
    sel_np = np.repeat(np.eye(4, dtype=np.float32), 32, axis=1)
    dw_np = np.ascontiguousarray(dense_w.astype(np.float32).reshape(2, 128, O))
    db_np = np.ascontiguousarray(dense_b.astype(np.float32).reshape(O, 1))

    in_maps = []
    for ci in range(NCORES):
        xc = x[ci * BL:(ci + 1) * BL, :t_steps, :]          # [BL, t, I]
        xt_np = np.ascontiguousarray(
            xc.transpose(2, 1, 0)).astype(bf)               # [I, t, BL]
        in_maps.append({
            names["xt"]: xt_np, names["wr"]: wr_np, names["kr"]: kr_np,
            names["b4"]: b4_np, names["sel"]: sel_np,
            names["dw"]: dw_np, names["db"]: db_np,
        })

    res = run_bass_kernel_spmd(nc, in_maps, core_ids=list(range(NCORES)),
                               trace=_want_trace)
    out = np.concatenate(
        [res.results[ci][names["yT"]].T for ci in range(NCORES)], axis=0)
    out = out.astype(np.float32)
    if _want_trace:
        return out, res
    return out



# revision 3
# speedup vs baseline: 1.2533x; 1.2533x over previous
"""JANet RNN kernel for Trainium2, 8-way data-parallel over batch.

Math (per batch row): with W = recurrent_kernel [C, 2C], b = recurrent_bias:
    z[t] = x[t] @ kernel + b + c[t-1] @ W
    f = sigmoid(z_f); g = tanh(z_g); c[t] = f*c + (1-f)*g
    out  = c[T-1] @ dense_w + dense_b

Device-side reparametrization (host-folded, exact up to bf16 weights):
    c = 2*ct + 1; master state stored negated n = -ct (fp32), n0 = +0.5
    u  = sigmoid(-z_f) = 1 - f ;  s = sigmoid(-2*z_g) = (1 - g)/2
    ehat = u*(n - s)  (= ct' - ct), n' = n - ehat
so the per-step activation is ONE sigmoid over the whole [128, 4x32] z tile
(signs/scales folded into effective weights), and the critical chain is just
MM -> sigmoid -> SUB -> MUL -> MM: the recurrent matmul consumes TWO bf16
streams (cbf = bf16(-n_{t-2}) and ehat_{t-1}) against the same stationary
W_eff, so the fp32 master update and the stream casts run off-chain.
b_eff uses colsum of the QUANTIZED W_eff so the c=2ct+1 constant cancels
exactly; the dense head runs in fp32 from the master (the output is the
precision-critical reduction).

Per-core layout (BL = 32 batch rows): state tiles [128, 2(ki), 32(b)] =
cell 128*ki+p; z PSUM window [128, 4(q banks), 512(16 steps x 32 b)] fp32,
double-buffered; per 16-step window one K=2 bias matmul (Dekker bf16 hi+lo
vs ones) + 4 projection matmuls (N=512) prime the window.
"""

import numpy as np
import ml_dtypes

B, T, I, C, O = 256, 1024, 128, 256, 10
NCORES = 8
BL = B // NCORES          # 32 batch rows per core
TW = 16                   # timesteps per PSUM z window (4 banks of N=512)

_cache: dict = {}


def _build(t_steps: int):
    import concourse.mybir as mybir
    from concourse import bacc, tile

    dt = mybir.dt
    fp32, bf16 = dt.float32, dt.bfloat16
    AF = mybir.ActivationFunctionType

    from contextlib import ExitStack

    assert t_steps % TW == 0
    nwin = t_steps // TW

    nc = bacc.Bacc(None, target_bir_lowering=False)
    names = {}

    with tile.TileContext(nc) as tc, ExitStack() as es:
        if True:
            dram = es.enter_context(tc.tile_pool(name="dram", bufs=1, space="DRAM"))
            xt = dram.tile([128, t_steps, BL], bf16, kind="ExternalInput", name="xt")
            we = dram.tile([2, 128, 2 * C], bf16, kind="ExternalInput", name="we")
            ke = dram.tile([128, 2 * C], bf16, kind="ExternalInput", name="ke")
            bhl = dram.tile([2, 2 * C], bf16, kind="ExternalInput", name="bhl")
            dwe = dram.tile([2, 128, O], fp32, kind="ExternalInput", name="dwe")
            dbe = dram.tile([O, 1], fp32, kind="ExternalInput", name="dbe")
            yT = dram.tile([O, BL], fp32, kind="ExternalOutput", name="yT")
            names = {k: v.tensor.name for k, v in
                     dict(xt=xt, we=we, ke=ke, bhl=bhl, dwe=dwe, dbe=dbe,
                          yT=yT).items()}

            cpool = es.enter_context(tc.tile_pool(name="consts", bufs=1))
            w0 = cpool.tile([128, 2 * C], bf16, tag="w0")
            w1 = cpool.tile([128, 2 * C], bf16, tag="w1")
            kes = cpool.tile([128, 2 * C], bf16, tag="kes")
            bhs = cpool.tile([2, 2 * C], bf16, tag="bhs")
            ones = cpool.tile([2, 512], bf16, tag="ones")
            dw0 = cpool.tile([128, O], fp32, tag="dw0")
            dw1 = cpool.tile([128, O], fp32, tag="dw1")
            dbs = cpool.tile([O, 1], fp32, tag="dbs")
            ysb = cpool.tile([O, BL], fp32, tag="ysb")

            nc.sync.dma_start(w0[:], we[0])
            nc.sync.dma_start(w1[:], we[1])
            nc.sync.dma_start(kes[:], ke[:])
            nc.sync.dma_start(bhs[:], bhl[:])
            nc.sync.dma_start(dw0[:], dwe[0])
            nc.sync.dma_start(dw1[:], dwe[1])
            nc.sync.dma_start(dbs[:], dbe[:])
            nc.vector.memset(ones[:], 1.0)

            xpool = es.enter_context(tc.tile_pool(name="xwin", bufs=3))
            zpool = es.enter_context(tc.tile_pool(name="zp", bufs=2, space="PSUM"))
            apool = es.enter_context(tc.tile_pool(name="ap", bufs=2))
            dpool = es.enter_context(tc.tile_pool(name="dp", bufs=2))
            epool = es.enter_context(tc.tile_pool(name="ep", bufs=2))
            npool = es.enter_context(tc.tile_pool(name="np", bufs=2))
            nbpool = es.enter_context(tc.tile_pool(name="nb", bufs=2))
            cbpool = es.enter_context(tc.tile_pool(name="cb", bufs=2))

            wtiles = (w0, w1)

            n32 = npool.tile([128, 2, BL], fp32, tag="n", name="n_init")
            nbf = nbpool.tile([128, 2, BL], bf16, tag="nb", name="nb_init")
            cbf = cbpool.tile([128, 2, BL], bf16, tag="cb", name="cb_init")
            eh = epool.tile([128, 2, BL], bf16, tag="e", name="e_init")
            nc.vector.memset(n32[:], 0.5)
            nc.vector.memset(nbf[:], 0.5)
            nc.vector.memset(cbf[:], -0.5)
            nc.vector.memset(eh[:], 0.0)

            def win_init(w):
                """bias + projection matmuls priming window w's PSUM tile."""
                zw = zpool.tile([128, 4, 512], fp32, tag="z", name=f"z{w}")
                xw = xpool.tile([128, TW * BL], bf16, tag="xw", name=f"xw{w}")
                nc.sync.dma_start(xw[:], xt[:, w * TW:(w + 1) * TW, :])
                for q in range(4):
                    nc.tensor.matmul(zw[:, q, :], bhs[:, 128 * q:128 * (q + 1)],
                                     ones[:], start=True, stop=False)
                for q in range(4):
                    nc.tensor.matmul(zw[:, q, :], kes[:, 128 * q:128 * (q + 1)],
                                     xw[:], start=False, stop=False)
                return zw

            zw_cur = win_init(0)
            for t in range(t_steps):
                tl = t % TW
                w = t // TW
                if tl == 1 and w + 1 < nwin:
                    zw_next = win_init(w + 1)

                # z[:, q, tl] += W_eff.T @ (cbf + eh)   (two bf16 streams)
                for ki in range(2):
                    for q in range(4):
                        zs = zw_cur[:, q, 32 * tl:32 * (tl + 1)]
                        wc = wtiles[ki][:, 128 * q:128 * (q + 1)]
                        nc.tensor.matmul(zs, wc, cbf[:, ki, :],
                                         start=False, stop=False)
                        nc.tensor.matmul(zs, wc, eh[:, ki, :],
                                         start=False, stop=(ki == 1 and q == 3))

                # one sigmoid over the whole step tile -> [u | s] bf16
                act = apool.tile([128, 4, BL], bf16, tag="a", name=f"a{t}")
                nc.scalar.activation(act[:], zw_cur[:, :, 32 * tl:32 * (tl + 1)],
                                     AF.Sigmoid)

                # chain: dd = n - s ; ehat = u*dd   (both bf16)
                dd = dpool.tile([128, 2, BL], bf16, tag="d", name=f"d{t}")
                ehn = epool.tile([128, 2, BL], bf16, tag="e", name=f"e{t}")
                nc.vector.tensor_sub(dd[:], nbf[:], act[:, 2:4, :])
                nc.vector.tensor_mul(ehn[:], dd[:], act[:, 0:2, :])

                # off-chain: stream base cast (reads OLD master), master
                # update, and bf16 shadow of the new master
                cbn = cbpool.tile([128, 2, BL], bf16, tag="cb", name=f"cb{t}")
                nn = npool.tile([128, 2, BL], fp32, tag="n", name=f"n{t}")
                nbn = nbpool.tile([128, 2, BL], bf16, tag="nb", name=f"nb{t}")
                nc.vector.tensor_scalar_mul(cbn[:], n32[:], -1.0)
                nc.vector.tensor_sub(nn[:], n32[:], ehn[:])
                nc.vector.tensor_copy(nbn[:], nn[:])
                n32, nbf, cbf, eh = nn, nbn, cbn, ehn
                if tl == TW - 1 and w + 1 < nwin:
                    zw_cur = zw_next

            # dense head in fp32 from the master: y = n32 @ (-2*dense_w) + dbe
            yp = zpool.tile([128, 4, 512], fp32, tag="z", name="yhead")
            nc.tensor.matmul(yp[0:O, 0, 0:BL], dw0[:], n32[:, 0, :],
                             start=True, stop=False)
            nc.tensor.matmul(yp[0:O, 0, 0:BL], dw1[:], n32[:, 1, :],
                             start=False, stop=True)
            nc.vector.tensor_scalar_add(ysb[:], yp[0:O, 0, 0:BL], dbs[:])
            nc.sync.dma_start(yT[:], ysb[:])

    nc.compile()
    return nc, names


def _get_built(t_steps: int):
    key = t_steps
    if key not in _cache:
        _cache[key] = _build(t_steps)
    return _cache[key]


def kernel(x, kernel, recurrent_kernel, recurrent_bias, dense_w, dense_b,
           t_steps: int = T, _want_trace: bool = False):
    from concourse.bass_utils import run_bass_kernel_spmd

    bf = ml_dtypes.bfloat16
    nc, names = _get_built(t_steps)

    W = recurrent_kernel.astype(np.float64)
    kern = kernel.astype(np.float64)
    b = recurrent_bias.astype(np.float64)

    w_eff = np.concatenate([-2.0 * W[:, :C], -4.0 * W[:, C:]], axis=1).astype(bf)
    k_eff = np.concatenate([-kern[:, :C], -2.0 * kern[:, C:]], axis=1).astype(bf)
    b_eff = (np.concatenate([-b[:C], -2.0 * b[C:]])
             + w_eff.astype(np.float64).sum(0) / 2).astype(np.float32)
    b_hi = b_eff.astype(bf)
    b_lo = (b_eff - b_hi.astype(np.float32)).astype(bf)

    we_np = np.ascontiguousarray(w_eff.reshape(2, 128, 2 * C))
    ke_np = np.ascontiguousarray(k_eff)
    bhl_np = np.ascontiguousarray(np.stack([b_hi, b_lo]))
    dwe_np = np.ascontiguousarray(
        (-2.0 * dense_w.astype(np.float64)).astype(np.float32)
        .reshape(2, 128, O))
    dbe_np = np.ascontiguousarray(
        (dense_b.astype(np.float64) + dense_w.astype(np.float64).sum(0))
        .astype(np.float32).reshape(O, 1))

    in_maps = []
    for ci in range(NCORES):
        xc = x[ci * BL:(ci + 1) * BL, :t_steps, :]          # [BL, t, I]
        xt_np = np.ascontiguousarray(
            xc.transpose(2, 1, 0)).astype(bf)               # [I, t, BL]
        in_maps.append({
            names["xt"]: xt_np, names["we"]: we_np, names["ke"]: ke_np,
            names["bhl"]: bhl_np, names["dwe"]: dwe_np, names["dbe"]: dbe_np,
        })

    res = run_bass_kernel_spmd(nc, in_maps, core_ids=list(range(NCORES)),
                               trace=_want_trace)
    out = np.concatenate(
        [res.results[ci][names["yT"]].T for ci in range(NCORES)], axis=0)
    out = out.astype(np.float32)
    if _want_trace:
        return out, res
    return out


# revision 6
# speedup vs baseline: 1.3973x; 1.1149x over previous
"""JANet RNN kernel for Trainium2, 8-way data-parallel over batch.

Math (per batch row): with W = recurrent_kernel [C, 2C], b = recurrent_bias:
    z[t] = x[t] @ kernel + b + c[t-1] @ W
    f = sigmoid(z_f); g = tanh(z_g); c[t] = f*c + (1-f)*g
    out  = c[T-1] @ dense_w + dense_b

Device-side reparametrization (host-folded, exact up to bf16 weights):
    c = 2*ct + 1; master state stored negated n = -ct (fp32), n0 = +0.5
    u  = sigmoid(-z_f) = 1 - f ;  s = sigmoid(-2*z_g) = (1 - g)/2
    ehat = u*(n - s)  (= ct' - ct), n' = n - ehat
so the per-step activation is ONE sigmoid over the whole [128, 4x32] z tile
(signs/scales folded into effective weights), and the critical chain is just
MM -> sigmoid -> SUB -> MUL -> MM: the recurrent matmul consumes TWO bf16
streams (cbf = bf16(-n_{t-2}) and ehat_{t-1}) against the same stationary
W_eff, so the fp32 master update and the stream casts run off-chain.
b_eff uses colsum of the QUANTIZED W_eff so the c=2ct+1 constant cancels
exactly; the dense head runs in fp32 from the master (the output is the
precision-critical reduction).

Per-core layout (BL = 32 batch rows): state tiles [128, 2(ki), 32(b)] =
cell 128*ki+p; z PSUM window [128, 4(q banks), 512(16 steps x 32 b)] fp32,
double-buffered; per 16-step window one K=2 bias matmul (Dekker bf16 hi+lo
vs ones) + 4 projection matmuls (N=512) prime the window.
"""

import numpy as np
import ml_dtypes

B, T, I, C, O = 256, 1024, 128, 256, 10
NCORES = 8
BL = B // NCORES          # 32 batch rows per core
TW = 16                   # timesteps per PSUM z window (4 banks of N=512)

_cache: dict = {}


def _build(t_steps: int):
    import concourse.mybir as mybir
    from concourse import bacc, tile

    dt = mybir.dt
    fp32, bf16 = dt.float32, dt.bfloat16
    AF = mybir.ActivationFunctionType

    from contextlib import ExitStack

    assert t_steps % TW == 0
    nwin = t_steps // TW

    nc = bacc.Bacc(None, target_bir_lowering=False)
    names = {}

    with tile.TileContext(nc) as tc, ExitStack() as es:
        if True:
            dram = es.enter_context(tc.tile_pool(name="dram", bufs=1, space="DRAM"))
            xt = dram.tile([128, t_steps, BL], bf16, kind="ExternalInput", name="xt")
            we = dram.tile([2, 128, 2 * C], bf16, kind="ExternalInput", name="we")
            ke = dram.tile([128, 2 * C], bf16, kind="ExternalInput", name="ke")
            bhl = dram.tile([2, 2 * C], bf16, kind="ExternalInput", name="bhl")
            dwe = dram.tile([2, 128, O], fp32, kind="ExternalInput", name="dwe")
            dbe = dram.tile([O, 1], fp32, kind="ExternalInput", name="dbe")
            yT = dram.tile([O, BL], fp32, kind="ExternalOutput", name="yT")
            names = {k: v.tensor.name for k, v in
                     dict(xt=xt, we=we, ke=ke, bhl=bhl, dwe=dwe, dbe=dbe,
                          yT=yT).items()}

            cpool = es.enter_context(tc.tile_pool(name="consts", bufs=1))
            w0 = cpool.tile([128, 2 * C], bf16, tag="w0")
            w1 = cpool.tile([128, 2 * C], bf16, tag="w1")
            kes = cpool.tile([128, 2 * C], bf16, tag="kes")
            bhs = cpool.tile([2, 2 * C], bf16, tag="bhs")
            ones = cpool.tile([2, 512], bf16, tag="ones")
            dw0 = cpool.tile([128, O], fp32, tag="dw0")
            dw1 = cpool.tile([128, O], fp32, tag="dw1")
            dbs = cpool.tile([O, 1], fp32, tag="dbs")
            ysb = cpool.tile([O, BL], fp32, tag="ysb")

            nc.sync.dma_start(w0[:], we[0])
            nc.sync.dma_start(w1[:], we[1])
            nc.sync.dma_start(kes[:], ke[:])
            nc.sync.dma_start(bhs[:], bhl[:])
            nc.sync.dma_start(dw0[:], dwe[0])
            nc.sync.dma_start(dw1[:], dwe[1])
            nc.sync.dma_start(dbs[:], dbe[:])
            nc.vector.memset(ones[:], 1.0)

            xpool = es.enter_context(tc.tile_pool(name="xwin", bufs=3))
            zpool = es.enter_context(tc.tile_pool(name="zp", bufs=2, space="PSUM"))
            apool = es.enter_context(tc.tile_pool(name="ap", bufs=2))
            dpool = es.enter_context(tc.tile_pool(name="dp", bufs=2))
            epool = es.enter_context(tc.tile_pool(name="ep", bufs=2))
            npool = es.enter_context(tc.tile_pool(name="np", bufs=2))
            cbpool = es.enter_context(tc.tile_pool(name="cb", bufs=2))

            wtiles = (w0, w1)

            n32 = npool.tile([128, 2, BL], fp32, tag="n", name="n_init")
            cbf = cbpool.tile([128, 2, BL], bf16, tag="cb", name="cb_init")
            eh = epool.tile([128, 2, BL], bf16, tag="e", name="e_init")
            nc.vector.memset(n32[:], 0.5)
            nc.vector.memset(cbf[:], -0.5)
            nc.vector.memset(eh[:], 0.0)

            def win_init(w):
                """bias + projection matmuls priming window w's PSUM tile."""
                zw = zpool.tile([128, 4, 512], fp32, tag="z", name=f"z{w}")
                xw = xpool.tile([128, TW * BL], bf16, tag="xw", name=f"xw{w}")
                nc.sync.dma_start(xw[:], xt[:, w * TW:(w + 1) * TW, :])
                for q in range(4):
                    nc.tensor.matmul(zw[:, q, :], bhs[:, 128 * q:128 * (q + 1)],
                                     ones[:], start=True, stop=False)
                for q in range(4):
                    nc.tensor.matmul(zw[:, q, :], kes[:, 128 * q:128 * (q + 1)],
                                     xw[:], start=False, stop=False)
                return zw

            zw_cur = win_init(0)
            for t in range(t_steps):
                tl = t % TW
                w = t // TW
                if tl == 1 and w + 1 < nwin:
                    zw_next = win_init(w + 1)

                # z[:, q, tl] += W_eff.T @ (cbf + eh): cbf-stream MMs first
                # (they prefire during the previous step's chain), g-half
                # chunks (q=2,3) first so ACT_g can start earliest.
                zs = lambda q: zw_cur[:, q, 32 * tl:32 * (tl + 1)]
                for q in (2, 3, 0, 1):
                    for ki in range(2):
                        nc.tensor.matmul(zs(q),
                                         wtiles[ki][:, 128 * q:128 * (q + 1)],
                                         cbf[:, ki, :], start=False, stop=False)
                for q in (2, 3, 0, 1):
                    for ki in range(2):
                        nc.tensor.matmul(zs(q),
                                         wtiles[ki][:, 128 * q:128 * (q + 1)],
                                         eh[:, ki, :], start=False,
                                         stop=(q == 1 and ki == 1))

                # split sigmoid: s-half first, u-half overlaps op1
                sg = apool.tile([128, 2, BL], bf16, tag="a", name=f"s{t}")
                uf = apool.tile([128, 2, BL], bf16, tag="a", name=f"u{t}")
                nc.scalar.activation(sg[:], zw_cur[:, 2:4, 32 * tl:32 * (tl + 1)],
                                     AF.Sigmoid)

                # chain: dd = n - s ; ehat = u*dd
                dd = dpool.tile([128, 2, BL], bf16, tag="d", name=f"d{t}")
                ehn = epool.tile([128, 2, BL], bf16, tag="e", name=f"e{t}")
                nc.vector.tensor_sub(dd[:], n32[:], sg[:])
                nc.scalar.activation(uf[:], zw_cur[:, 0:2, 32 * tl:32 * (tl + 1)],
                                     AF.Sigmoid)
                nc.vector.tensor_mul(ehn[:], dd[:], uf[:])

                # off-chain: stream base cast (reads OLD master) + master update
                cbn = cbpool.tile([128, 2, BL], bf16, tag="cb", name=f"cb{t}")
                nn = npool.tile([128, 2, BL], fp32, tag="n", name=f"n{t}")
                nc.vector.tensor_scalar_mul(cbn[:], n32[:], -1.0)
                nc.vector.tensor_sub(nn[:], n32[:], ehn[:])
                n32, cbf, eh = nn, cbn, ehn
                if tl == TW - 1 and w + 1 < nwin:
                    zw_cur = zw_next

            # dense head in fp32 from the master: y = n32 @ (-2*dense_w) + dbe
            yp = zpool.tile([128, 4, 512], fp32, tag="z", name="yhead")
            nc.tensor.matmul(yp[0:O, 0, 0:BL], dw0[:], n32[:, 0, :],
                             start=True, stop=False)
            nc.tensor.matmul(yp[0:O, 0, 0:BL], dw1[:], n32[:, 1, :],
                             start=False, stop=True)
            nc.vector.tensor_scalar_add(ysb[:], yp[0:O, 0, 0:BL], dbs[:])
            nc.sync.dma_start(yT[:], ysb[:])

    nc.compile()
    return nc, names


def _get_built(t_steps: int):
    key = t_steps
    if key not in _cache:
        _cache[key] = _build(t_steps)
    return _cache[key]


def kernel(x, kernel, recurrent_kernel, recurrent_bias, dense_w, dense_b,
           t_steps: int = T, _want_trace: bool = False):
    from concourse.bass_utils import run_bass_kernel_spmd

    bf = ml_dtypes.bfloat16
    nc, names = _get_built(t_steps)

    W = recurrent_kernel.astype(np.float64)
    kern = kernel.astype(np.float64)
    b = recurrent_bias.astype(np.float64)

    w_eff = np.concatenate([-2.0 * W[:, :C], -4.0 * W[:, C:]], axis=1).astype(bf)
    k_eff = np.concatenate([-kern[:, :C], -2.0 * kern[:, C:]], axis=1).astype(bf)
    b_eff = (np.concatenate([-b[:C], -2.0 * b[C:]])
             + w_eff.astype(np.float64).sum(0) / 2).astype(np.float32)
    b_hi = b_eff.astype(bf)
    b_lo = (b_eff - b_hi.astype(np.float32)).astype(bf)

    we_np = np.ascontiguousarray(w_eff.reshape(2, 128, 2 * C))
    ke_np = np.ascontiguousarray(k_eff)
    bhl_np = np.ascontiguousarray(np.stack([b_hi, b_lo]))
    dwe_np = np.ascontiguousarray(
        (-2.0 * dense_w.astype(np.float64)).astype(np.float32)
        .reshape(2, 128, O))
    dbe_np = np.ascontiguousarray(
        (dense_b.astype(np.float64) + dense_w.astype(np.float64).sum(0))
        .astype(np.float32).reshape(O, 1))

    in_maps = []
    for ci in range(NCORES):
        xc = x[ci * BL:(ci + 1) * BL, :t_steps, :]          # [BL, t, I]
        xt_np = np.ascontiguousarray(
            xc.transpose(2, 1, 0)).astype(bf)               # [I, t, BL]
        in_maps.append({
            names["xt"]: xt_np, names["we"]: we_np, names["ke"]: ke_np,
            names["bhl"]: bhl_np, names["dwe"]: dwe_np, names["dbe"]: dbe_np,
        })

    res = run_bass_kernel_spmd(nc, in_maps, core_ids=list(range(NCORES)),
                               trace=_want_trace)
    out = np.concatenate(
        [res.results[ci][names["yT"]].T for ci in range(NCORES)], axis=0)
    out = out.astype(np.float32)
    if _want_trace:
        return out, res
    return out


# revision 9
# speedup vs baseline: 1.4473x; 1.0358x over previous
"""JANet RNN kernel for Trainium2, 8-way data-parallel over batch.

Math (per batch row): with W = recurrent_kernel [C, 2C], b = recurrent_bias:
    z[t] = x[t] @ kernel + b + c[t-1] @ W
    f = sigmoid(z_f); g = tanh(z_g); c[t] = f*c + (1-f)*g
    out  = c[T-1] @ dense_w + dense_b

Device-side reparametrization (host-folded, exact up to bf16 weights):
    c = 2*ct + 1; master state stored negated n = -ct (fp32), n0 = +0.5
    u  = sigmoid(-z_f) = 1 - f ;  s = sigmoid(-2*z_g) = (1 - g)/2
    ehat = u*(n - s)  (= ct' - ct), n' = n - ehat
so the per-step activation is ONE sigmoid over the whole [128, 4x32] z tile
(signs/scales folded into effective weights), and the critical chain is just
MM -> sigmoid -> SUB -> MUL -> MM: the recurrent matmul consumes TWO bf16
streams (cbf = bf16(-n_{t-2}) and ehat_{t-1}) against the same stationary
W_eff, so the fp32 master update and the stream casts run off-chain.
b_eff uses colsum of the QUANTIZED W_eff so the c=2ct+1 constant cancels
exactly; the dense head runs in fp32 from the master (the output is the
precision-critical reduction).

Per-core layout (BL = 32 batch rows): state tiles [128, 2(ki), 32(b)] =
cell 128*ki+p; z PSUM window [128, 4(q banks), 512(16 steps x 32 b)] fp32,
double-buffered; per 16-step window one K=2 bias matmul (Dekker bf16 hi+lo
vs ones) + 4 projection matmuls (N=512) prime the window.
"""

import numpy as np
import ml_dtypes

B, T, I, C, O = 256, 1024, 128, 256, 10
NCORES = 8
BL = B // NCORES          # 32 batch rows per core
TW = 16                   # timesteps per PSUM z window (4 banks of N=512)

_cache: dict = {}


def _build(t_steps: int):
    import concourse.mybir as mybir
    from concourse import bacc, tile

    dt = mybir.dt
    fp32, bf16 = dt.float32, dt.bfloat16
    AF = mybir.ActivationFunctionType

    from contextlib import ExitStack

    assert t_steps % TW == 0
    nwin = t_steps // TW

    nc = bacc.Bacc(None, target_bir_lowering=False)
    names = {}

    with tile.TileContext(nc) as tc, ExitStack() as es:
        if True:
            dram = es.enter_context(tc.tile_pool(name="dram", bufs=1, space="DRAM"))
            xt = dram.tile([128, t_steps, BL], bf16, kind="ExternalInput", name="xt")
            we = dram.tile([2, 128, 2 * C], bf16, kind="ExternalInput", name="we")
            ke = dram.tile([128, 2 * C], bf16, kind="ExternalInput", name="ke")
            bhl = dram.tile([2, 2 * C], bf16, kind="ExternalInput", name="bhl")
            dwe = dram.tile([2, 128, O], fp32, kind="ExternalInput", name="dwe")
            dbe = dram.tile([O, 1], fp32, kind="ExternalInput", name="dbe")
            yT = dram.tile([O, BL], fp32, kind="ExternalOutput", name="yT")
            names = {k: v.tensor.name for k, v in
                     dict(xt=xt, we=we, ke=ke, bhl=bhl, dwe=dwe, dbe=dbe,
                          yT=yT).items()}

            cpool = es.enter_context(tc.tile_pool(name="consts", bufs=1))
            w0 = cpool.tile([128, 2 * C], bf16, tag="w0")
            w1 = cpool.tile([128, 2 * C], bf16, tag="w1")
            kes = cpool.tile([128, 2 * C], bf16, tag="kes")
            bhs = cpool.tile([2, 2 * C], bf16, tag="bhs")
            ones = cpool.tile([2, 512], bf16, tag="ones")
            dw0 = cpool.tile([128, O], fp32, tag="dw0")
            dw1 = cpool.tile([128, O], fp32, tag="dw1")
            dbs = cpool.tile([O, 1], fp32, tag="dbs")
            ysb = cpool.tile([O, BL], fp32, tag="ysb")

            nc.sync.dma_start(w0[:], we[0])
            nc.sync.dma_start(w1[:], we[1])
            nc.sync.dma_start(kes[:], ke[:])
            nc.sync.dma_start(bhs[:], bhl[:])
            nc.sync.dma_start(dw0[:], dwe[0])
            nc.sync.dma_start(dw1[:], dwe[1])
            nc.sync.dma_start(dbs[:], dbe[:])
            nc.vector.memset(ones[:], 1.0)

            xpool = es.enter_context(tc.tile_pool(name="xwin", bufs=3))
            zpool = es.enter_context(tc.tile_pool(name="zp", bufs=2, space="PSUM"))
            apool = es.enter_context(tc.tile_pool(name="ap", bufs=2))
            dpool = es.enter_context(tc.tile_pool(name="dp", bufs=2))
            epool = es.enter_context(tc.tile_pool(name="ep", bufs=2))
            npool = es.enter_context(tc.tile_pool(name="np", bufs=2))
            cbpool = es.enter_context(tc.tile_pool(name="cb", bufs=2))

            wtiles = (w0, w1)

            n32 = npool.tile([128, 2, BL], fp32, tag="n", name="n_init")
            cbf = cbpool.tile([128, 2, BL], bf16, tag="cb", name="cb_init")
            eh = epool.tile([128, 2, BL], bf16, tag="e", name="e_init")
            nc.vector.memset(n32[:], 0.5)
            nc.vector.memset(cbf[:], -0.5)
            nc.vector.memset(eh[:], 0.0)

            def win_init(w, lazy):
                """bias + projection matmuls priming window w's PSUM tile.

                When lazy, returns (zw, thunks): the 8 matmuls are emitted one
                per step (inside the ACT/DVE window of the running step) so a
                500 ns N=512 matmul never lands in front of chain matmuls.
                """
                zw = zpool.tile([128, 4, 512], fp32, tag="z", name=f"z{w}")
                xw = xpool.tile([128, TW * BL], bf16, tag="xw", name=f"xw{w}")
                nc.sync.dma_start(xw[:], xt[:, w * TW:(w + 1) * TW, :])
                thunks = []
                for q in range(4):
                    thunks.append(lambda q=q: nc.tensor.matmul(
                        zw[:, q, :], bhs[:, 128 * q:128 * (q + 1)],
                        ones[:], start=True, stop=False))
                for q in range(4):
                    thunks.append(lambda q=q: nc.tensor.matmul(
                        zw[:, q, :], kes[:, 128 * q:128 * (q + 1)],
                        xw[:], start=False, stop=False))
                if not lazy:
                    for th in thunks:
                        th()
                    thunks = []
                return zw, thunks

            zw_cur, _ = win_init(0, lazy=False)
            pending = []
            for t in range(t_steps):
                tl = t % TW
                w = t // TW
                if tl == 0 and w + 1 < nwin:
                    zw_next, pending = win_init(w + 1, lazy=True)

                # z[:, q, tl] += W_eff.T @ (cbf + eh): cbf-stream MMs first
                # (they prefire during the previous step's chain), g-half
                # chunks (q=2,3) first so ACT_g can start earliest.
                zs = lambda q: zw_cur[:, q, 32 * tl:32 * (tl + 1)]
                for q in (2, 3, 0, 1):
                    for ki in range(2):
                        nc.tensor.matmul(zs(q),
                                         wtiles[ki][:, 128 * q:128 * (q + 1)],
                                         cbf[:, ki, :], start=False, stop=False)
                for q in (2, 3, 0, 1):
                    for ki in range(2):
                        nc.tensor.matmul(zs(q),
                                         wtiles[ki][:, 128 * q:128 * (q + 1)],
                                         eh[:, ki, :], start=False,
                                         stop=(q == 1 and ki == 1))
                if pending:
                    pending.pop(0)()

                # split sigmoid: s-half first, u-half overlaps op1
                sg = apool.tile([128, 2, BL], bf16, tag="a", name=f"s{t}")
                uf = apool.tile([128, 2, BL], bf16, tag="a", name=f"u{t}")
                nc.scalar.activation(sg[:], zw_cur[:, 2:4, 32 * tl:32 * (tl + 1)],
                                     AF.Sigmoid)

                # chain: dd = n - s ; ehat = u*dd
                dd = dpool.tile([128, 2, BL], bf16, tag="d", name=f"d{t}")
                ehn = epool.tile([128, 2, BL], bf16, tag="e", name=f"e{t}")
                nc.vector.tensor_sub(dd[:], n32[:], sg[:])
                nc.scalar.activation(uf[:], zw_cur[:, 0:2, 32 * tl:32 * (tl + 1)],
                                     AF.Sigmoid)
                nc.vector.tensor_mul(ehn[:], dd[:], uf[:])

                # off-chain: stream base cast (reads OLD master) + master update
                cbn = cbpool.tile([128, 2, BL], bf16, tag="cb", name=f"cb{t}")
                nn = npool.tile([128, 2, BL], fp32, tag="n", name=f"n{t}")
                nc.vector.tensor_scalar_mul(cbn[:], n32[:], -1.0)
                nc.gpsimd.tensor_sub(nn[:], n32[:], ehn[:])
                n32, cbf, eh = nn, cbn, ehn
                if tl == TW - 1 and w + 1 < nwin:
                    zw_cur = zw_next

            # dense head in fp32 from the master: y = n32 @ (-2*dense_w) + dbe
            yp = zpool.tile([128, 4, 512], fp32, tag="z", name="yhead")
            nc.tensor.matmul(yp[0:O, 0, 0:BL], dw0[:], n32[:, 0, :],
                             start=True, stop=False)
            nc.tensor.matmul(yp[0:O, 0, 0:BL], dw1[:], n32[:, 1, :],
                             start=False, stop=True)
            nc.vector.tensor_scalar_add(ysb[:], yp[0:O, 0, 0:BL], dbs[:])
            nc.sync.dma_start(yT[:], ysb[:])

    nc.compile()
    return nc, names


def _get_built(t_steps: int):
    key = t_steps
    if key not in _cache:
        _cache[key] = _build(t_steps)
    return _cache[key]


def kernel(x, kernel, recurrent_kernel, recurrent_bias, dense_w, dense_b,
           t_steps: int = T, _want_trace: bool = False):
    from concourse.bass_utils import run_bass_kernel_spmd

    bf = ml_dtypes.bfloat16
    nc, names = _get_built(t_steps)

    W = recurrent_kernel.astype(np.float64)
    kern = kernel.astype(np.float64)
    b = recurrent_bias.astype(np.float64)

    w_eff = np.concatenate([-2.0 * W[:, :C], -4.0 * W[:, C:]], axis=1).astype(bf)
    k_eff = np.concatenate([-kern[:, :C], -2.0 * kern[:, C:]], axis=1).astype(bf)
    b_eff = (np.concatenate([-b[:C], -2.0 * b[C:]])
             + w_eff.astype(np.float64).sum(0) / 2).astype(np.float32)
    b_hi = b_eff.astype(bf)
    b_lo = (b_eff - b_hi.astype(np.float32)).astype(bf)

    we_np = np.ascontiguousarray(w_eff.reshape(2, 128, 2 * C))
    ke_np = np.ascontiguousarray(k_eff)
    bhl_np = np.ascontiguousarray(np.stack([b_hi, b_lo]))
    dwe_np = np.ascontiguousarray(
        (-2.0 * dense_w.astype(np.float64)).astype(np.float32)
        .reshape(2, 128, O))
    dbe_np = np.ascontiguousarray(
        (dense_b.astype(np.float64) + dense_w.astype(np.float64).sum(0))
        .astype(np.float32).reshape(O, 1))

    in_maps = []
    for ci in range(NCORES):
        xc = x[ci * BL:(ci + 1) * BL, :t_steps, :]          # [BL, t, I]
        xt_np = np.ascontiguousarray(
            xc.transpose(2, 1, 0)).astype(bf)               # [I, t, BL]
        in_maps.append({
            names["xt"]: xt_np, names["we"]: we_np, names["ke"]: ke_np,
            names["bhl"]: bhl_np, names["dwe"]: dwe_np, names["dbe"]: dbe_np,
        })

    res = run_bass_kernel_spmd(nc, in_maps, core_ids=list(range(NCORES)),
                               trace=_want_trace)
    out = np.concatenate(
        [res.results[ci][names["yT"]].T for ci in range(NCORES)], axis=0)
    out = out.astype(np.float32)
    if _want_trace:
        return out, res
    return out
